# revision 6
# baseline (speedup 1.0000x reference)
"""DenseEnergyLoss on 8 Trainium2 NeuronCores (Bass/Tile) — v3.

Reference: per image, Wm = exp(-0.5*d2(f_p,f_q)) over 5-dim features
f = (x/sxy, y/sxy, rgb/15), loss = -W/N * sum(S * ((S @ Wm) * gate)).
P = 64*64 pixels after the 0.5-scale resizes (host-side, O(P)).

Device formulation (v3, transposed R-form + dual-engine exp):
  Wm is symmetric; each core owns 16 parity-interleaved q-blocks (128 rows)
  of one image (2 cores/image) and computes only tiles with p <= q:
  tile (local i, p-chunk d) = exp(X)[q-block, p-cols], q-block j = 2i+h.

  X tiles are produced by a K<=32 matmul of feature vectors; since K<=32,
  the two 512-col halves of each G tile run CONCURRENTLY on different
  32-row strips of the PE array (tile_position row tiling, outputs in
  different PSUM banks).

  exp is split across TWO engines (the scalar engine alone is the
  bottleneck otherwise):
   - ACT tiles: scalar-engine exp (exact, fp16 out), error-compensated
     hi/lo fp16 features (K=21, zero-padded to 32).
   - DVE tiles: Schraudolph exp — the G matmul computes A*X directly
     (features pre-scaled by sqrt(A), A = 1024*log2(e)); one vector-engine
     tensor_scalar((A*X + B) max 0) -> int16, bitcast to fp16, approximates
     exp(X) with ~3% zero-mean sawtooth error (fine for the 2e-2 gate;
     validated 1.6e-3 end-to-end in simulation).

  Second matmul (R-form): R[p,k] += Wm'[q,p]^T @ [S|SG]^T[q,k] with the
  Wm piece as the 128x128 stationary operand and only 42 streamed cols,
  accumulated over q-blocks in a 1-bank PSUM accumulator per chunk.
  loss partial = sum_p,k R[p,k] * [SG|S]^T[p,k] via ONE fused DVE
  tensor_tensor_reduce per chunk (chained accumulator).

  Diagonal-band tiles mask the 256-col window holding the diagonal with
  a host-built {0,1,0.5} mask (parity baked into mask DATA so the program
  is identical across cores).

Host does only O(P) prep: resizes, gating, feature build, sharding, and
the final sum of 8 per-core [128] partials.
"""

import numpy as np

# problem shapes (hardcoded per contract)
N_IMG = 4
K = 21
H = 128
W = 128
HO, WO = 64, 64
P = HO * WO            # 4096
NCH = 4                # p-chunks of 1024
QCH = 1024
N_CORES = 8

SIGMA_RGB = 15.0
SXY = 100.0 * 0.5      # SIGMA_XY * SCALE
WEIGHT = 1e-7

A_SCH = 1024.0 / np.log(2.0)          # Schraudolph scale (2^10 * log2 e)
SQA = float(np.sqrt(A_SCH))
B_DELTA = 0.0407                       # zero-mean sawtooth correction
B_SCH = float(15.0 * 1024 - 1024 * B_DELTA + 0.5)   # +0.5: convert truncates

DVE_FULL = 15          # of the 24 full tiles per core, how many use DVE exp

_CACHE = {}


def _full_tile_dve_set():
    """Evenly spread DVE_FULL of 24 full tiles (Bresenham)."""
    return {fi for fi in range(24)
            if (fi * DVE_FULL) // 24 != ((fi + 1) * DVE_FULL) // 24}


def _build_module(loop_n=1):
    from contextlib import ExitStack

    import concourse.bacc as bacc
    import concourse.tile as tile
    from concourse import mybir

    fp32 = mybir.dt.float32
    fp16 = mybir.dt.float16
    i16 = mybir.dt.int16

    nc = bacc.Bacc(trn_type="TRN2", target_bir_lowering=False, debug=False)

    FEAT = nc.declare_dram_parameter("FEAT", [128, P], fp16, isOutput=False)
    VFEAT = nc.declare_dram_parameter("VFEAT", [128, 2048], fp16, isOutput=False)
    STG = nc.declare_dram_parameter("STG", [128, 16 * 42], fp16, isOutput=False)
    SGSTD = nc.declare_dram_parameter("SGSTD", [128, 4 * 336], fp16,
                                      isOutput=False)
    MASK = nc.declare_dram_parameter("MASK", [128, 256], fp16, isOutput=False)
    OUT = nc.declare_dram_parameter("out", [128, 1], fp32, isOutput=True)

    dve_set = _full_tile_dve_set()

    with tile.TileContext(nc) as tc, ExitStack() as ctx:
        singles = ctx.enter_context(tc.tile_pool(name="singles", bufs=1))
        gpool = ctx.enter_context(tc.tile_pool(name="g", bufs=3, space="PSUM"))
        rpool = ctx.enter_context(tc.tile_pool(name="r", bufs=2, space="PSUM"))
        wpool = ctx.enter_context(tc.tile_pool(name="wm", bufs=32))
        spool = ctx.enter_context(tc.tile_pool(name="scr", bufs=2))
        apool = ctx.enter_context(tc.tile_pool(name="acc", bufs=5))

        # DMAs ordered by first use.
        sb_VF = singles.tile([128, 2048], fp16)
        nc.sync.dma_start(out=sb_VF, in_=VFEAT.ap())
        sb_FE = singles.tile([128, P], fp16)
        nc.sync.dma_start(out=sb_FE[:, 0:QCH], in_=FEAT.ap()[:, 0:QCH])
        sb_MASK = singles.tile([128, 256], fp16)
        nc.sync.dma_start(out=sb_MASK, in_=MASK.ap())
        sb_STG = singles.tile([128, 16 * 42], fp16)
        nc.sync.dma_start(out=sb_STG, in_=STG.ap())
        for cc in range(1, NCH):
            nc.sync.dma_start(out=sb_FE[:, cc * QCH:(cc + 1) * QCH],
                              in_=FEAT.ap()[:, cc * QCH:(cc + 1) * QCH])
        sb_SGS = singles.tile([128, 4 * 336], fp16)
        nc.sync.dma_start(out=sb_SGS, in_=SGSTD.ap())

        def body():
            fi = 0                       # full-tile counter
            cols = apool.tile([128, NCH], fp32)
            for d in range(NCH):
                R = rpool.tile([128, 512], fp32)   # pieces at 64-stride, 42 used
                wms = []                 # (i, w, wm) produced this chunk
                for i in range(4 * d, 16):
                    band = i < 4 * d + 4
                    w = 256 * (i - 4 * d + 1) if band else QCH
                    if band:
                        use_dve = False
                    else:
                        use_dve = fi in dve_set
                        fi += 1
                    # feature row ranges: ACT hi/lo at 0/32, Schraudolph at 64/96
                    r0, r1 = (64, 96) if use_dve else (0, 32)
                    G = gpool.tile([128, QCH], fp32)
                    # G matmul: two concurrent 32-row strips in different banks
                    pieces = [(0, min(512, w), r0)]
                    if w > 512:
                        pieces.append((512, w - 512, r1))
                    for (o, wp, rr) in pieces:
                        nc.tensor.matmul(
                            G[:, o:o + wp],
                            lhsT=sb_VF[rr:rr + 32, 128 * i:128 * (i + 1)],
                            rhs=sb_FE[rr:rr + 32, QCH * d + o:QCH * d + o + wp],
                            start=True, stop=True, skip_group_check=True,
                            tile_position=(rr, 0),
                        )
                    wt = wpool.tile([128, QCH], fp16)
                    if use_dve:
                        nc.vector.tensor_scalar(
                            out=wt.bitcast(i16)[:, 0:w], in0=G[:, 0:w],
                            scalar1=B_SCH, scalar2=0.0,
                            op0=mybir.AluOpType.add, op1=mybir.AluOpType.max,
                        )
                    else:
                        nc.scalar.activation(
                            out=wt[:, 0:w], in_=G[:, 0:w],
                            func=mybir.ActivationFunctionType.Exp,
                        )
                    if band:
                        nc.vector.tensor_tensor(
                            out=wt[:, w - 256:w], in0=wt[:, w - 256:w],
                            in1=sb_MASK, op=mybir.AluOpType.mult,
                        )
                    wms.append((i, w, wt))
                # R accumulation: one CONTIGUOUS matmul group per piece
                # (interleaved groups in one PSUM bank break has_written).
                for pp in range(8):
                    contrib = [(i, wm) for (i, w, wm) in wms
                               if w >= 128 * (pp + 1)]
                    for ci, (i, wm) in enumerate(contrib):
                        nc.tensor.matmul(
                            R[:, 64 * pp:64 * pp + 42],
                            lhsT=wm[:, 128 * pp:128 * (pp + 1)],
                            rhs=sb_STG[:, 42 * i:42 * (i + 1)],
                            start=(ci == 0), stop=(ci == len(contrib) - 1),
                            skip_group_check=True,
                        )
                scr = spool.tile([128, 336], fp32)
                nc.vector.tensor_tensor(
                    out=scr.rearrange("p (n c) -> p n c", n=8),
                    in0=R.rearrange("p (n c) -> p n c", n=8)[:, :, 0:42],
                    in1=sb_SGS[:, 336 * d:336 * (d + 1)].rearrange(
                        "p (n c) -> p n c", n=8),
                    op=mybir.AluOpType.mult,
                )
                nc.vector.reduce_sum(out=cols[:, d:d + 1], in_=scr,
                                     axis=mybir.AxisListType.X)
            acc = apool.tile([128, 1], fp32)
            nc.vector.reduce_sum(out=acc, in_=cols, axis=mybir.AxisListType.X)
            return acc

        if loop_n == 1:
            acc = body()
        else:
            with tc.For_i(0, loop_n) as _:
                acc = body()
        nc.sync.dma_start(out=OUT.ap(), in_=acc)

    nc.compile()
    return nc


def get_module(loop_n=1):
    key = ("nc", loop_n)
    if key not in _CACHE:
        _CACHE[key] = _build_module(loop_n)
    return _CACHE[key]


def preprocess(images, segmentations, ROIs, seg_label):
    """Host-side O(P) prep: resizes, gating, features, per-core sharding."""
    images = np.asarray(images, dtype=np.float32)
    seg = np.asarray(segmentations, dtype=np.float32)
    roi = np.asarray(ROIs, dtype=np.float32)
    lbl = np.asarray(seg_label)

    img_s = images[:, :, ::2, ::2]                    # nearest resize x0.5
    roi_s = roi[:, ::2, ::2]
    lbl_s = lbl[:, :, ::2, ::2]
    seg_s = 0.25 * (seg[:, :, ::2, ::2] + seg[:, :, 1::2, ::2]
                    + seg[:, :, ::2, 1::2] + seg[:, :, 1::2, 1::2])

    unlabel = (lbl_s == 255)[:, 0]
    gate = np.maximum(
        np.where(unlabel, np.float32(1.0), roi_s - seg_s.max(axis=1)), 0.0
    ).astype(np.float32)
    S = (seg_s * roi_s[:, None]).reshape(N_IMG, K, P).astype(np.float32)
    SG = (S * gate.reshape(N_IMG, 1, P)).astype(np.float32)

    yy, xx = np.meshgrid(np.arange(HO, dtype=np.float32),
                         np.arange(WO, dtype=np.float32), indexing="ij")
    pos = np.stack([xx.ravel() / SXY, yy.ravel() / SXY], axis=-1)  # [P,2]

    tri = (np.tril(np.ones((128, 128), np.float32), -1)
           + 0.5 * np.eye(128, dtype=np.float32)).astype(np.float16)
    masks = []
    m0 = np.zeros((128, 256), np.float16)
    m0[:, 0:128] = tri
    masks.append(m0)
    m1 = np.ones((128, 256), np.float16)
    m1[:, 128:256] = tri
    masks.append(m1)

    in_maps = []
    for n in range(N_IMG):
        col = img_s[n].reshape(3, P).T / SIGMA_RGB
        f = np.concatenate([pos, col], axis=-1).astype(np.float32)  # [P,5]
        sq = np.sum(f * f, axis=-1)
        ones = np.ones((P, 1), np.float32)
        # ACT features: error-compensated hi/lo fp16, K=21
        u = np.concatenate([f, -0.5 * sq[:, None], ones], axis=1)   # [P,7]
        v = np.concatenate([f, ones, -0.5 * sq[:, None]], axis=1)
        uh = u.astype(np.float16)
        ul = (u - uh.astype(np.float32)).astype(np.float16)
        vh = v.astype(np.float16)
        vl = (v - vh.astype(np.float32)).astype(np.float16)
        U21 = np.concatenate([uh, uh, ul], axis=1)                  # [P,21]
        V21 = np.concatenate([vh, vl, vh], axis=1)
        # DVE features: single fp16 scaled by sqrt(A), K=7
        u2 = np.concatenate([SQA * f, -0.5 * SQA * sq[:, None],
                             np.full((P, 1), SQA, np.float32)], axis=1)
        v2 = np.concatenate([SQA * f, np.full((P, 1), SQA, np.float32),
                             -0.5 * SQA * sq[:, None]], axis=1)
        u2 = u2.astype(np.float16)
        v2 = v2.astype(np.float16)

        FEAT = np.zeros((128, P), np.float16)
        FEAT[0:21] = U21.T
        FEAT[32:53] = U21.T
        FEAT[64:71] = u2.T
        FEAT[96:103] = u2.T

        ST = S[n].T.astype(np.float16)                              # [P,21]
        SGT = SG[n].T.astype(np.float16)

        for h in range(2):
            VFEAT = np.zeros((128, 2048), np.float16)
            STGm = np.zeros((128, 16 * 42), np.float16)
            for i in range(16):
                j = 2 * i + h
                qs = slice(128 * j, 128 * (j + 1))
                VFEAT[0:21, 128 * i:128 * (i + 1)] = V21[qs].T
                VFEAT[32:53, 128 * i:128 * (i + 1)] = V21[qs].T
                VFEAT[64:71, 128 * i:128 * (i + 1)] = v2[qs].T
                VFEAT[96:103, 128 * i:128 * (i + 1)] = v2[qs].T
                STGm[:, 42 * i:42 * i + 21] = ST[qs]
                STGm[:, 42 * i + 21:42 * i + 42] = SGT[qs]
            SGSTD = np.zeros((128, 4 * 336), np.float16)
            for d in range(4):
                for pp in range(8):
                    ps = slice(1024 * d + 128 * pp, 1024 * d + 128 * (pp + 1))
                    o = 336 * d + 42 * pp
                    SGSTD[:, o:o + 21] = SGT[ps]
                    SGSTD[:, o + 21:o + 42] = ST[ps]
            in_maps.append({
                "FEAT": FEAT,
                "VFEAT": VFEAT,
                "STG": STGm,
                "SGSTD": SGSTD,
                "MASK": masks[h],
            })
    return in_maps


def kernel(images, segmentations, ROIs, seg_label):
    from concourse.bass_utils import run_bass_kernel_spmd

    nc = get_module()
    in_maps = preprocess(images, segmentations, ROIs, seg_label)
    res = run_bass_kernel_spmd(nc, in_maps, list(range(N_CORES)))
    total = 0.0
    for r in res.results:
        total += float(np.asarray(r["out"], dtype=np.float64).sum())
    return np.array([-WEIGHT * total / N_IMG], dtype=np.float32)
